# revision 62
# baseline (speedup 1.0000x reference)
"""TRN2 Bass kernel for nn_CardClassifier: CNN(4x conv3x3+relu+maxpool2) ->
per-feature sigmoid attention -> 128 stacked expert MLPs -> fusion MLP.

Sharding: pure data parallel. 8 cores x 4 images, weights replicated.
Single kernel launch per core, no collectives.

Structure (two phases, each pinned to its own roofline):
- conv phase (PE-bound): conv1 runs bf16 with a host-side ky-replicated
  padded input (K=36, kx in the free dim -> 3 passes instead of 9) split
  across two PE row-tiles (base partitions 0/64); conv2 (K=32/img, image
  pairs) and conv3 (K=64/img) also run as two concurrent row-tiles;
  conv4 is full K=128. PSUM is used as 2x [128,4,512] multi-bank tiles so
  each relu drains 4 banks in one Activation op. The attention DVE chain
  is emitted inside the conv4 loop so it overlaps remaining conv work.
- expert/fusion phase (DMA-bound): all weights stream over HWDGE (nc.sync)
  in 1MB-class chunks; B-chunks (K=69) and e4 are host-padded to K=128
  (zero rows + bias row) so transfers use all partitions and FWL engages;
  moving rows 69..127 of the T tiles are zero-filled once. The first two
  e1 weight groups prefetch during the convs (conv2's input A1 is bf16,
  which freed the SBUF for the second head group; only SBUF headroom
  limits deeper prefetch); fusion fw1/fw2/fw3 chunks are merged 4-per-DMA.

Cost-model history: baseline 841us (Pool/SWDGE-bound) -> 517 (HWDGE conv
inputs) -> 409 (HWDGE weights, G=32) -> 388 (buffering, merged fusion
DMAs) -> 375 (bf16 conv1, deferred prefetch, DMA ordering) -> 365
(bf16 A1/conv2 + second prefetch-head group) -> 363us (expert A/B
quadrant DMAs merged: one [128, 32x196] transfer per group instead of
four) -> 354us (third prefetch-head group: 9.4MB of e1 weights now
stream during the convs). Relative error 0.0125 (gate 2e-2); the bf16
conv1/conv2 steps cost 0.0095 -> 0.0125.
"""

import sys

sys.path.insert(0, "/opt/trn_rl_repo")

import json as _json
import numpy as np
import ml_dtypes

import concourse.bass as bass
import concourse.mybir as mybir
from concourse import tile
from concourse.bass_utils import run_bass_kernel_spmd

F32 = mybir.dt.float32
F32R = mybir.dt.float32r
BF16 = mybir.dt.bfloat16
AF = mybir.ActivationFunctionType
ALU = None  # filled lazily

B, CIN, H, W = 32, 3, 224, 224
NCORES, BL = 8, 4  # 4 images per core
CHANS = [3, 32, 64, 128, 128]
NF, FLAT = 128, 196
EXP_DIMS = [196, 196, 196, 98, 24, 16]
FIN_DIMS = [128 * 16, 2038, 2028, 53]

# dtype knobs
EW_NP = ml_dtypes.bfloat16   # expert weights + activations
FW_NP = ml_dtypes.bfloat16   # fusion weights + activations
EW_DT, FW_DT = BF16, BF16
XR_DT = BF16
XR_NP = ml_dtypes.bfloat16

_BUILT = None  # cached nc
DEBUG = False


# ---------------------------------------------------------------- tilefix
def _fix_bir_json(raw: bytes) -> bytes:
    """This walrus build allows at most 1 sync-wait per instruction; Tile's
    tail drain can carry more. Split extras onto NoOp carriers (same engine,
    inserted just before, so stream order semantics are unchanged)."""
    d = _json.loads(raw)
    k = 0
    for fn in d.get("functions", []):
        for blk in fn.get("blocks", []):
            out = []
            for inst in blk["instructions"]:
                si = inst.get("sync_info")
                waits = (si or {}).get("on_wait") or []
                if len(waits) > 1:
                    for wchunk in waits[:-1]:
                        out.append({
                            "debug": inst.get("debug", 0),
                            "engine": inst["engine"],
                            "ins": [], "outs": [],
                            "name": f"NOPW-{k}",
                            "opcode": "NoOp",
                            "sync_info": {"on_update": [], "on_wait": [wchunk]},
                        })
                        k += 1
                    si["on_wait"] = waits[-1:]
                out.append(inst)
            blk["instructions"] = out
    return _json.dumps(d).encode()


# ---------------------------------------------------------------- build
def _build():
    global ALU
    from concourse.alu_op_type import AluOpType as ALU_

    ALU = ALU_
    nc = bass.Bass("TRN2", target_bir_lowering=False, debug=False)

    dp = lambda name, shape, dt: nc.declare_dram_parameter(name, list(shape), dt, isOutput=False)

    # conv1 input: host-padded, ky-replicated: [36=(3ky,4img,3ch), 224, 226]
    xr_in = dp("xr", [36, H, W + 2], XR_DT)
    ident_in = dp("ident", [128, 128], F32R)
    cw1_in = dp("cwr0", [128, 3, 128], XR_DT)   # rows (ky,img,ch) at base 0 and 64
    cw_in = [None, dp("cwr1", [128, 9, 128], XR_DT),
             dp("cwr2", [128, 9, 128], F32R), dp("cwr3", [128, 9, 128], F32R)]
    cb_in = [dp(f"cbr{i}", [128, 1], F32) for i in range(4)]
    aw_in = dp("awr", [128, FLAT], F32)
    ab_in = dp("abr", [128, 1], F32)

    # expert weights, K-major chunked [K, 128e, ochunk], bias appended to B rows
    ew_shapes = {
        "e1A": (128, 128, 196), "e1B": (128, 128, 196),
        "e2A": (128, 128, 196), "e2B": (128, 128, 196),
        "e3AB": (128, 128, 2, 98),
        "e4": (128, 128, 24),
        "e5": (32, 128, 16),
    }
    ew_in = {k: dp(k, list(s), EW_DT) for k, s in ew_shapes.items()}

    # fusion weights: fw1 rows permuted on host to match F0 layout
    fw1_in = dp("fw1p", [16, 128, FIN_DIMS[1]], FW_DT)   # [slice, K=128, 2038]
    fb1_in = dp("fb1r", [1, FIN_DIMS[1]], FW_DT)
    fw2_in = dp("fw2r", [FIN_DIMS[1], FIN_DIMS[2]], FW_DT)
    fb2_in = dp("fb2r", [1, FIN_DIMS[2]], FW_DT)
    fw3_in = dp("fw3r", [FIN_DIMS[2], FIN_DIMS[3]], FW_DT)
    fb3_in = dp("fb3r", [1, FIN_DIMS[3]], FW_DT)
    ones_in = dp("onesrow", [1, 512], EW_DT)
    onespad_in = dp("onespad", [32, 512], EW_DT)

    y_out = nc.declare_dram_parameter("y", [BL, 53], F32, isOutput=True)
    if DEBUG:
        dbg = {
            "dbg_feats": nc.declare_dram_parameter("dbg_feats", [BL, 128, FLAT], F32, isOutput=True),
            "dbg_ta": nc.declare_dram_parameter("dbg_ta", [128, 512], EW_DT, isOutput=True),
            "dbg_tb": nc.declare_dram_parameter("dbg_tb", [69, 512], EW_DT, isOutput=True),
            "dbg_t2a": nc.declare_dram_parameter("dbg_t2a", [128, 512], EW_DT, isOutput=True),
            "dbg_t3a": nc.declare_dram_parameter("dbg_t3a", [128, 512], EW_DT, isOutput=True),
            "dbg_o5": nc.declare_dram_parameter("dbg_o5", [16, 512], FW_DT, isOutput=True),
            "dbg_fsb": nc.declare_dram_parameter("dbg_fsb", [128, 64], FW_DT, isOutput=True),
            "dbg_s1": nc.declare_dram_parameter("dbg_s1", [128, 64], F32, isOutput=True),
            "dbg_s2": nc.declare_dram_parameter("dbg_s2", [128, 64], F32, isOutput=True),
        }

    r32 = lambda ap: ap.bitcast(F32R)

    with tile.TileContext(nc, pool_alloc_mode="queue") as tc:
        import contextlib

        stk = contextlib.ExitStack()
        with stk:
            # ---- persistent pools (small constants only)
            wpool = stk.enter_context(tc.tile_pool(name="wconst", bufs=1))
            cw1sb = wpool.tile([128, 3, 128], XR_DT, name="cwsb0")
            nc.gpsimd.dma_start(cw1sb[:], cw1_in[:])
            cb = []
            for i in range(4):
                t = wpool.tile([128, 1], F32, name=f"cbsb{i}")
                nc.gpsimd.dma_start(t[:], cb_in[i][:])
                cb.append(t)
            cw = [None]
            for i in range(1, 4):
                t = wpool.tile([128, 9, 128], XR_DT if i == 1 else F32R, name=f"cwsb{i}")
                nc.gpsimd.dma_start(t[:], cw_in[i][:])
                cw.append(t)
            awsb = wpool.tile([128, FLAT], F32)
            nc.gpsimd.dma_start(awsb[:], aw_in[:])
            absb = wpool.tile([128, 1], F32)
            nc.gpsimd.dma_start(absb[:], ab_in[:])
            ident = wpool.tile([128, 128], F32R)
            nc.gpsimd.dma_start(ident[:], ident_in[:])

            featpool = stk.enter_context(tc.tile_pool(name="feats", bufs=1))
            feats = [featpool.tile([128, FLAT], F32R, name=f"feats{i}") for i in range(BL)]

            hs = [featpool.tile([128, FLAT], F32R, name=f"hs{i}") for i in range(BL)]
            tpool = stk.enter_context(tc.tile_pool(name="texp", bufs=1))
            # T1 in [i, e*4+img] layout, built by the inline attention chain
            Ta = tpool.tile([128, 512], EW_DT)
            Tb = tpool.tile([128, 512], EW_DT)
            nc.gpsimd.dma_start(Tb[68:96, :], onespad_in[0:28, :])
            nc.vector.memset(Tb[96:128, :], 0.0)

            G = 32  # experts per weight-DMA group
            # prefetch head: first expert-weight group, streamed during convs
            # (DMAs issued on the SP ring after conv1's input loads)
            eheadpool = stk.enter_context(tc.tile_pool(name="ehead", bufs=1))
            eheads = [{}, {}, {}]
            for hg in range(3):
                for sfx in ("A", "B"):
                    eheads[hg][sfx] = eheadpool.tile([128, G * 196], EW_DT, name=f"e1h{hg}" + sfx)

            import contextlib as _ctx
            actstk = _ctx.ExitStack()
            apool = actstk.enter_context(tc.tile_pool(name="acts", bufs=1))

            # =========================================================
            # conv1: 3->32, 224x224. K=36 (ky-replicated input, 4 imgs
            # diagonal-packed); kx in free dim (3 passes). Two row-tiles
            # at PE row groups 0/64 process consecutive 8-row strips.
            # =========================================================
            A1p = apool.tile([128, 114 * 114], XR_DT)
            a1v = A1p.rearrange("p (r c) -> p r c", c=114)
            nc.vector.memset(a1v[:, 0, :], 0.0)
            nc.vector.memset(a1v[:, 113, :], 0.0)
            nc.vector.memset(a1v[:, :, 0], 0.0)
            nc.vector.memset(a1v[:, :, 113], 0.0)

            with tc.tile_pool(name="psc", bufs=2, space="PSUM") as psum_cv, \
                 tc.tile_pool(name="c1x", bufs=3) as c1xpool, \
                 tc.tile_pool(name="c1o", bufs=2) as c1opool, \
                 tc.tile_pool(name="c1v", bufs=2) as c1vpool:
                for rnd in range(14):
                    xt = c1xpool.tile([128, 8 * 226], XR_DT, tag="xr")
                    xtv = xt.rearrange("p (r c) -> p r c", c=226)
                    for ti in range(2):
                        base = 64 * ti
                        r0 = 16 * rnd + 8 * ti
                        nc.sync.dma_start(xtv[base:base + 36, :, :], xr_in[:, r0:r0 + 8, :])
                    for ti in range(2):
                        base = 64 * ti
                        r0 = 16 * rnd + 8 * ti
                        o1 = c1opool.tile([128, 8 * 224], F32R, tag="ob")
                        o1v = o1.rearrange("p (r c) -> p r c", c=224)
                        P = psum_cv.tile([128, 4, 512], F32, tag="acc")
                        for b in range(4):
                            for kx in range(3):
                                rhs = xtv[base:base + 36, 2 * b:2 * b + 2, kx:kx + 224]
                                nc.tensor.matmul(
                                    P[:, b, 0:448], cw1sb[base:base + 36, kx, :], rhs,
                                    start=(kx == 0), stop=(kx == 2))
                        nc.scalar.activation(
                            o1.rearrange("p (b c) -> p b c", c=448)[:],
                            P[:, :, 0:448], AF.Relu, bias=cb[0][:])
                        pv = c1vpool.tile([128, 4 * 224], F32R, tag="pvb")
                        pvv = pv.rearrange("p (r c) -> p r c", c=224)
                        nc.vector.tensor_tensor(pvv[:], o1v[:, 0:8:2, :], o1v[:, 1:8:2, :], op=ALU.max)
                        j0 = r0 // 2
                        nc.vector.tensor_tensor(
                            a1v[:, 1 + j0:1 + j0 + 4, 1:113],
                            pvv[:, :, 0:224:2], pvv[:, :, 1:224:2], op=ALU.max,
                        )

                for hg in range(3):
                    for sfx in ("A", "B"):
                        nc.sync.dma_start(
                            eheads[hg][sfx].rearrange("k (e o) -> k e o", o=196)[:],
                            ew_in["e1" + sfx][:, hg * G:(hg + 1) * G, :])

                # =========================================================
                # conv2: 32->64, 112x112, K=32 per img; both img pairs run
                # concurrently as row-tiles 0/64. 4 PSUM banks per tile.
                # =========================================================
                A2p = [apool.tile([128, 58 * 58], F32R, name=f"A2p{i}") for i in range(2)]
                for p2 in range(2):
                    a2v = A2p[p2].rearrange("p (r c) -> p r c", c=58)
                    nc.vector.memset(a2v[:, 0, :].bitcast(F32), 0.0)
                    nc.vector.memset(a2v[:, 57, :].bitcast(F32), 0.0)
                    nc.vector.memset(a2v[:, :, 0].bitcast(F32), 0.0)
                    nc.vector.memset(a2v[:, :, 57].bitcast(F32), 0.0)

                a1vv = A1p.rearrange("p (r c) -> p r c", c=114)
                for rnd in range(7):     # 16 out rows per tile per round
                    for pr in range(2):
                        base = 64 * pr
                        a2v = A2p[pr].rearrange("p (r c) -> p r c", c=58)
                        o2 = c1opool.tile([128, 16 * 112], F32R, tag="ob")
                        o2v = o2.rearrange("p (r c) -> p r c", c=112)
                        P = psum_cv.tile([128, 4, 512], F32, tag="acc")
                        for b in range(4):
                            t = 4 * rnd + b
                            for k in range(9):
                                ky, kx = divmod(k, 3)
                                rhs = a1vv[base:base + 64, 4 * t + ky:4 * t + ky + 4, kx:kx + 112]
                                nc.tensor.matmul(
                                    P[:, b, 0:448], cw[1][base:base + 64, k, :], rhs,
                                    start=(k == 0), stop=(k == 8))
                        nc.scalar.activation(
                            o2.rearrange("p (b c) -> p b c", c=448)[:],
                            P[:, :, 0:448], AF.Relu, bias=cb[1][:])
                        pv = c1vpool.tile([128, 8 * 112], F32R, tag="pvb")
                        pvv = pv.rearrange("p (r c) -> p r c", c=112)
                        nc.vector.tensor_tensor(pvv[:], o2v[:, 0:16:2, :], o2v[:, 1:16:2, :], op=ALU.max)
                        nc.vector.tensor_tensor(
                            a2v[:, 1 + 8 * rnd:1 + 8 * rnd + 8, 1:57],
                            pvv[:, :, 0:112:2], pvv[:, :, 1:112:2], op=ALU.max)

                # =========================================================
                # conv3: 64->128, 56x56, K=64; imgs within a pair run as
                # row-tiles 0/64, pairs serial. 8-row chunks (448 cols).
                # =========================================================
                A3p = [apool.tile([128, 30 * 30], F32R, name=f"A3p{i}") for i in range(BL)]
                for img in range(BL):
                    a3v = A3p[img].rearrange("p (r c) -> p r c", c=30)
                    nc.vector.memset(a3v[:, 0, :].bitcast(F32), 0.0)
                    nc.vector.memset(a3v[:, 29, :].bitcast(F32), 0.0)
                    nc.vector.memset(a3v[:, :, 0].bitcast(F32), 0.0)
                    nc.vector.memset(a3v[:, :, 29].bitcast(F32), 0.0)

                for pr in range(2):
                    a2v = A2p[pr].rearrange("p (r c) -> p r c", c=58)
                    for rnd in range(2):
                        nb = 4 if rnd == 0 else 3
                        for sl in range(2):
                            base = 64 * sl
                            img = 2 * pr + sl
                            a3v = A3p[img].rearrange("p (r c) -> p r c", c=30)
                            o3 = c1opool.tile([128, nb * 8 * 56], F32R, tag="ob")
                            o3v = o3.rearrange("p (r c) -> p r c", c=56)
                            P = psum_cv.tile([128, 4, 512], F32, tag="acc")
                            for b in range(nb):
                                t = 4 * rnd + b
                                for k in range(9):
                                    ky, kx = divmod(k, 3)
                                    rhs = a2v[base:base + 64, 8 * t + ky:8 * t + ky + 8, kx:kx + 56]
                                    nc.tensor.matmul(
                                        P[:, b, 0:448], cw[2][base:base + 64, k, :], rhs,
                                        start=(k == 0), stop=(k == 8))
                            nc.scalar.activation(
                                o3.rearrange("p (b c) -> p b c", c=448)[:],
                                P[:, 0:nb, 0:448], AF.Relu, bias=cb[2][:])
                            pv = c1vpool.tile([128, nb * 4 * 56], F32R, tag="pvb")
                            pvv = pv.rearrange("p (r c) -> p r c", c=56)
                            nc.vector.tensor_tensor(pvv[:], o3v[:, 0:nb * 8:2, :], o3v[:, 1:nb * 8:2, :], op=ALU.max)
                            nc.vector.tensor_tensor(
                                a3v[:, 1 + 16 * rnd:1 + 16 * rnd + 4 * nb, 1:29],
                                pvv[:, :, 0:56:2], pvv[:, :, 1:56:2], op=ALU.max)

                # =========================================================
                # conv4: 128->128, 28x28, K=128, serial per img; the
                # attention chain for each img is emitted inline so it
                # overlaps the remaining images' conv4 work
                # =========================================================
                for img in range(BL):
                    a3v = A3p[img].rearrange("p (r c) -> p r c", c=30)
                    fv = feats[img].rearrange("p (r c) -> p r c", c=14)
                    o4 = c1opool.tile([128, 28 * 28], F32R, tag="ob")
                    o4v = o4.rearrange("p (r c) -> p r c", c=28)
                    P = psum_cv.tile([128, 2, 512], F32, tag="acc")
                    for t in range(2):   # 14 rows x 28 = 392
                        for k in range(9):
                            ky, kx = divmod(k, 3)
                            rhs = a3v[:, 14 * t + ky:14 * t + ky + 14, kx:kx + 28]
                            nc.tensor.matmul(
                                P[:, t, 0:392], cw[3][:, k, :], rhs,
                                start=(k == 0), stop=(k == 8))
                    nc.scalar.activation(
                        o4.rearrange("p (b c) -> p b c", c=392)[:],
                        P[:, :, 0:392], AF.Relu, bias=cb[3][:])
                    pv = c1vpool.tile([128, 14 * 28], F32R, tag="pvb")
                    pvv = pv.rearrange("p (r c) -> p r c", c=28)
                    nc.vector.tensor_tensor(pvv[:], o4v[:, 0:28:2, :], o4v[:, 1:28:2, :], op=ALU.max)
                    nc.vector.tensor_tensor(
                        fv[:], pvv[:, :, 0:28:2], pvv[:, :, 1:28:2], op=ALU.max)

                    # attention + h = feats*att (DVE/ACT chain overlaps
                    # the remaining conv4 PE work; transposes come after)
                    tmp = c1vpool.tile([128, FLAT], F32, tag="tmp")
                    nc.vector.tensor_tensor(tmp[:], feats[img][:], awsb[:], op=ALU.mult)
                    attv = c1vpool.tile([128, 1], F32, tag="attv")
                    nc.vector.tensor_reduce(attv[:], tmp[:], axis=mybir.AxisListType.X, op=ALU.add)
                    atts = c1vpool.tile([128, 1], F32, tag="atts")
                    nc.scalar.activation(atts[:], attv[:], AF.Sigmoid, bias=absb[:])
                    nc.vector.tensor_scalar(hs[img][:], feats[img][:], atts[:, 0:1], None, op0=ALU.mult)
            actstk.close()

            epool = stk.enter_context(tc.tile_pool(name="ew", bufs=4))
            fpool = stk.enter_context(tc.tile_pool(name="fw", bufs=2))
            psum_c = stk.enter_context(tc.tile_pool(name="psacc", bufs=5, space="PSUM"))
            psum_t = stk.enter_context(tc.tile_pool(name="pstr", bufs=2, space="PSUM"))

            # transpose h -> T1 layout [i, e*4+img]
            for img in range(BL):
                PT = psum_t.tile([128, 128], F32R, tag="tr")
                nc.tensor.transpose(PT[:], hs[img][:, 0:128], ident[:])
                nc.vector.tensor_copy(Ta.rearrange("p (e i) -> p e i", i=4)[:, :, img], PT[:])
                PT2 = psum_t.tile([128, 128], F32R, tag="tr")
                nc.tensor.transpose(PT2[0:68, :], hs[img][:, 128:196], ident[:])
                nc.vector.tensor_copy(Tb.rearrange("p (e i) -> p e i", i=4)[0:68, :, img], PT2[0:68, :])

            if DEBUG:
                for img in range(BL):
                    nc.gpsimd.dma_start(dbg["dbg_feats"][img], feats[img][:].bitcast(F32))
                nc.gpsimd.dma_start(dbg["dbg_ta"][:], Ta[:])
                nc.gpsimd.dma_start(dbg["dbg_tb"][:], Tb[:])

            # =========================================================
            # experts: 5 layers, stationary=ew (bf16+FWL), moving=T slices
            # outputs land directly in next layer's T layout
            # =========================================================
            psum_e = psum_c

            # L1/L2: in 196 (A128+B69), out 196 (A128 + B68)
            def full_layer(Tin_a, Tin_b, pre, head=None):
                PA = psum_e.tile([128, 512], F32, tag="acc")
                PB = psum_e.tile([68, 512], F32, tag="acc")
                for g in range(128 // G):
                    if head is not None and g < len(head):
                        wts = head[g]
                    else:
                        wts = {}
                        for sfx in ("A", "B"):
                            t = epool.tile([128, G * 196], EW_DT, tag="ew", name=pre + sfx + "t")
                            nc.sync.dma_start(
                                t.rearrange("k (e o) -> k e o", o=196)[:],
                                ew_in[pre + sfx][:, g * G:(g + 1) * G, :])
                            wts[sfx] = t
                    tAv = wts["A"].rearrange("k (e o) -> k e o", o=196)
                    tBv = wts["B"].rearrange("k (e o) -> k e o", o=196)
                    for j in range(G):
                        e = g * G + j
                        sl = slice(4 * e, 4 * e + 4)
                        nc.tensor.matmul(PA[:, sl], tAv[:, j, 0:128],
                                         Tin_a[:, sl], start=True, stop=False)
                        nc.tensor.matmul(PA[:, sl], tBv[:, j, 0:128],
                                         Tin_b[:, sl], start=False, stop=True)
                        nc.tensor.matmul(PB[:, sl], tAv[:, j, 128:196],
                                         Tin_a[:, sl], start=True, stop=False)
                        nc.tensor.matmul(PB[:, sl], tBv[:, j, 128:196],
                                         Tin_b[:, sl], start=False, stop=True)
                Toa = tpool.tile([128, 512], EW_DT, name=pre + "oa")
                Tob = tpool.tile([128, 512], EW_DT, name=pre + "ob")
                nc.scalar.activation(Toa[:], PA[:], AF.Relu)
                nc.scalar.activation(Tob[0:68, :], PB[:], AF.Relu)
                nc.gpsimd.dma_start(Tob[68:96, :], onespad_in[0:28, :])
                nc.vector.memset(Tob[96:128, :], 0.0)
                return Toa, Tob

            T2a, T2b = full_layer(Ta, Tb, "e1", head=eheads)
            T3a, T3b = full_layer(T2a, T2b, "e2")
            if DEBUG:
                nc.gpsimd.dma_start(dbg["dbg_t2a"][:], T2a[:])
                nc.gpsimd.dma_start(dbg["dbg_t3a"][:], T3a[:])

            # L3: in 196, out 98
            P98 = psum_e.tile([98, 512], F32, tag="acc")
            for g in range(128 // G):
                tAB = epool.tile([128, G * 2 * 98], EW_DT, tag="ew")
                tv = tAB.rearrange("k (e t o) -> k e t o", t=2, o=98)
                nc.sync.dma_start(tv[:], ew_in["e3AB"][:, g * G:(g + 1) * G, :, :])
                for j in range(G):
                    e = g * G + j
                    sl = slice(4 * e, 4 * e + 4)
                    nc.tensor.matmul(P98[:, sl], tv[:, j, 0, :], T3a[:, sl], start=True, stop=False)
                    nc.tensor.matmul(P98[:, sl], tv[:, j, 1, :], T3b[:, sl], start=False, stop=True)
            T4 = tpool.tile([128, 512], EW_DT)
            nc.scalar.activation(T4[0:98, :], P98[:], AF.Relu)
            nc.gpsimd.dma_start(T4[98:128, :], onespad_in[0:30, :])

            # L4: in 98(+1), out 24
            P24 = psum_e.tile([24, 512], F32, tag="acc")
            for g in range(128 // G):
                t4 = epool.tile([128, G * 24], EW_DT, tag="ew")
                nc.sync.dma_start(t4.rearrange("k (e o) -> k e o", o=24)[:], ew_in["e4"][:, g * G:(g + 1) * G, :])
                for j in range(G):
                    e = g * G + j
                    sl = slice(4 * e, 4 * e + 4)
                    nc.tensor.matmul(P24[:, sl], t4.rearrange("k (e o) -> k e o", o=24)[:, j, :], T4[:, sl], start=True, stop=True)
            T5 = tpool.tile([32, 512], EW_DT)
            nc.scalar.activation(T5[0:24, :], P24[:], AF.Relu)
            nc.gpsimd.dma_start(T5[24:32, :], onespad_in[0:8, :])

            # L5: in 24(+1), out 16; final relu (reference relus the stack output)
            P16 = psum_e.tile([16, 512], F32, tag="acc")
            for g in range(128 // G):
                t5 = epool.tile([32, G * 16], EW_DT, tag="ew")
                nc.sync.dma_start(t5.rearrange("k (e o) -> k e o", o=16)[:], ew_in["e5"][:, g * G:(g + 1) * G, :])
                for j in range(G):
                    e = g * G + j
                    sl = slice(4 * e, 4 * e + 4)
                    nc.tensor.matmul(P16[:, sl], t5.rearrange("k (e o) -> k e o", o=16)[:, j, :], T5[:, sl], start=True, stop=True)
            O5 = tpool.tile([16, 512], FW_DT)   # [o, e*4+img]
            nc.scalar.activation(O5[:], P16[:], AF.Relu)

            # ---- Fsb [128e, (img,o)=64]: 4 img-strided transposes of O5
            identb = wpool.tile([128, 128], FW_DT)
            nc.vector.tensor_copy(identb[:], ident[:])
            PT5 = psum_t.tile([128, 64], FW_DT, tag="tr")
            O5v = O5.rearrange("o (e i) -> o e i", i=BL)
            for img in range(BL):
                nc.tensor.transpose(PT5[:, 16 * img:16 * img + 16], O5v[:, :, img], identb[0:16, 0:16])
            Fsb = tpool.tile([128, 64], FW_DT)  # [e, img*16+o]
            nc.vector.tensor_copy(Fsb[:], PT5[:])
            if DEBUG:
                nc.gpsimd.dma_start(dbg["dbg_o5"][:], O5[:])
                nc.gpsimd.dma_start(dbg["dbg_fsb"][:], Fsb[:])

            # =========================================================
            # fusion: stationary = fw chunks (bf16+FWL), moving = [K,4img]
            # layer outputs land as SxT [n%128, mc*4+img] = next moving form
            # =========================================================
            psum_f = psum_c
            ones1 = wpool.tile([1, BL], FW_DT)
            nc.vector.memset(ones1[:], 1.0)

            def nchunks(d):
                return (d + 127) // 128

            # fw1: contraction k=(e,o): 16 o-planes x K=128e; moving = Fsb[:, o::16]
            D1 = FIN_DIMS[1]
            nmc1 = nchunks(D1)   # 16
            S1T = tpool.tile([128, 4 * nmc1], F32)
            P1f = psum_f.tile([128, 4 * nmc1], F32, tag="acc")
            fb1t = fpool.tile([1, D1], FW_DT, tag="fb", bufs=3)
            nc.sync.dma_start(fb1t[:], fb1_in[:])
            Fsbv = Fsb.rearrange("e (i o) -> e i o", o=16)
            for op in range(4):
                w1t = fpool.tile([128, 4, D1], FW_DT, tag="fw")
                nc.sync.dma_start(w1t[:], fw1_in[4 * op:4 * op + 4, :, :].rearrange("o k c -> k o c"))
                for oo in range(4):
                    o = 4 * op + oo
                    mov = Fsbv[:, :, o]          # [128e, 4img] stride 16
                    for mc in range(nmc1):
                        m0, m1 = 128 * mc, min(128 * (mc + 1), D1)
                        nc.tensor.matmul(P1f[0:m1 - m0, 4 * mc:4 * mc + 4], w1t[:, oo, m0:m1], mov,
                                         start=(o == 0 and mc == 0), stop=False)
            for mc in range(nmc1):
                m0, m1 = 128 * mc, min(128 * (mc + 1), D1)
                nc.tensor.matmul(P1f[0:m1 - m0, 4 * mc:4 * mc + 4], fb1t[:, m0:m1], ones1[:],
                                 start=False, stop=(mc == nmc1 - 1))
            nc.scalar.activation(S1T[:], P1f[:], AF.Relu)
            if DEBUG:
                nc.gpsimd.dma_start(dbg["dbg_s1"][:], S1T[:])
            S1b = tpool.tile([128, 4 * nmc1], FW_DT)
            nc.vector.tensor_copy(S1b[:], S1T[:])

            # fw2: straightforward 16kc x 16mc
            D2 = FIN_DIMS[2]
            nmc2 = nchunks(D2)
            S2T = tpool.tile([128, 4 * nmc2], F32)
            P2f = psum_f.tile([128, 4 * nmc2], F32, tag="acc")
            fb2t = fpool.tile([1, D2], FW_DT, tag="fb", bufs=3)
            nc.sync.dma_start(fb2t[:], fb2_in[:])
            for kp in range(4):
                # 4 K-chunks per DMA: [128, 4, D2] via rearrange; ragged tail split off
                kws = [(128 * kc, min(128 * (kc + 1), D1)) for kc in range(4 * kp, min(4 * kp + 4, nchunks(D1)))]
                nfull = sum(1 for k0, k1 in kws if k1 - k0 == 128)
                w2t = fpool.tile([128, 4, D2], FW_DT, tag="fw")
                if nfull:
                    nc.sync.dma_start(
                        w2t[:, 0:nfull, :],
                        fw2_in[kws[0][0]:kws[0][0] + 128 * nfull, :].rearrange("(kc p) c -> p kc c", p=128))
                for ci, (k0, k1) in enumerate(kws[nfull:], start=nfull):
                    nc.sync.dma_start(w2t[0:k1 - k0, ci, :], fw2_in[k0:k1, :])
                for ci, (k0, k1) in enumerate(kws):
                    kc = 4 * kp + ci
                    mov = S1b[0:k1 - k0, 4 * kc:4 * kc + 4]
                    for mc in range(nmc2):
                        m0, m1 = 128 * mc, min(128 * (mc + 1), D2)
                        nc.tensor.matmul(P2f[0:m1 - m0, 4 * mc:4 * mc + 4], w2t[0:k1 - k0, ci, m0:m1], mov,
                                         start=(kc == 0 and mc == 0), stop=False)
            for mc in range(nmc2):
                m0, m1 = 128 * mc, min(128 * (mc + 1), D2)
                nc.tensor.matmul(P2f[0:m1 - m0, 4 * mc:4 * mc + 4], fb2t[:, m0:m1], ones1[:],
                                 start=False, stop=(mc == nmc2 - 1))
            nc.scalar.activation(S2T[:], P2f[:], AF.Relu)
            if DEBUG:
                nc.gpsimd.dma_start(dbg["dbg_s2"][:], S2T[:])
            S2b = tpool.tile([128, 4 * nmc2], FW_DT)
            nc.vector.tensor_copy(S2b[:], S2T[:])

            # fw3 -> [53, 4]
            D3 = FIN_DIMS[3]
            P3f = psum_f.tile([53, BL], F32, tag="acc")
            fb3t = fpool.tile([1, D3], FW_DT, tag="fb", bufs=3)
            nc.sync.dma_start(fb3t[:], fb3_in[:])
            nk3 = nchunks(D2)     # 16 chunks; 15 full + 108-row tail
            w3t = fpool.tile([128, nk3, D3], FW_DT, tag="fw")
            nc.sync.dma_start(
                w3t[:, 0:nk3 - 1, :],
                fw3_in[0:128 * (nk3 - 1), :].rearrange("(kc p) c -> p kc c", p=128))
            nc.sync.dma_start(w3t[0:D2 - 128 * (nk3 - 1), nk3 - 1, :], fw3_in[128 * (nk3 - 1):D2, :])
            for kc in range(nk3):
                k0, k1 = 128 * kc, min(128 * (kc + 1), D2)
                nc.tensor.matmul(P3f[:], w3t[0:k1 - k0, kc, :], S2b[0:k1 - k0, 4 * kc:4 * kc + 4],
                                 start=(kc == 0), stop=False)
            nc.tensor.matmul(P3f[:], fb3t[:], ones1[:], start=False, stop=True)
            S3 = tpool.tile([53, BL], F32)
            nc.scalar.activation(S3[:], P3f[:], AF.Copy)
            nc.gpsimd.dma_start(y_out[:].rearrange("b o -> o b"), S3[:])

    # tilefix patch
    orig = nc.to_json_bytes
    nc.to_json_bytes = lambda: _fix_bir_json(orig())
    return nc


def ew_shapes_cw(i):
    return [128, 9, 128]


# ---------------------------------------------------------------- host prep
def _host_prep(inputs):
    f = lambda a: np.asarray(a, dtype=np.float32)
    cws = [f(inputs[f"cw{i+1}"]) for i in range(4)]
    cbs = [f(inputs[f"cb{i+1}"]) for i in range(4)]
    base = {}
    base["ident"] = np.eye(128, dtype=np.float32)

    # conv1 stationary [128, 3(kx), 128]: rows (ky, img, ch) replicated at
    # base 0 and 64 (two PE row-tiles); cols (img, oc) block-diagonal
    t = np.zeros((128, 3, 128), np.float32)
    # lhs1[ch, ky, kx, oc]
    lhs1 = cws[0].transpose(1, 2, 3, 0)
    for bb in (0, 64):
        for ky in range(3):
            for img in range(4):
                t[bb + 12 * ky + 3 * img:bb + 12 * ky + 3 * img + 3, :, 32 * img:32 * img + 32] = \
                    lhs1[:, ky, :, :]
    base["cwr0"] = t.astype(XR_NP)
    # cw2: K=64 (2img x 32ch) block-diag: rows 32s+c -> cols 64s+o; replicated per pair
    t = np.zeros((128, 9, 128), np.float32)
    lhs2 = cws[1].transpose(1, 2, 3, 0).reshape(32, 9, 64)
    for s in range(2):
        t[32 * s:32 * s + 32, :, 64 * s:64 * s + 64] = lhs2
    t[64:128] = t[0:64]
    base["cwr1"] = t.astype(XR_NP)
    # cw3: K=64 per img at rows 64sl
    t = np.zeros((128, 9, 128), np.float32)
    lhs3 = cws[2].transpose(1, 2, 3, 0).reshape(64, 9, 128)
    t[0:64] = lhs3
    t[64:128] = lhs3
    base["cwr2"] = t
    # cw4: K=128
    base["cwr3"] = np.ascontiguousarray(cws[3].transpose(1, 2, 3, 0).reshape(128, 9, 128))
    # conv biases
    cbr = []
    for i, cbi in enumerate(cbs):
        t = np.zeros((128, 1), np.float32)
        if i == 0:
            for img in range(BL):
                t[32 * img:32 * img + 32, 0] = cbi
        elif i == 1:
            for s in range(2):
                t[64 * s:64 * s + 64, 0] = cbi
        else:
            t[:, 0] = cbi
        base[f"cbr{i}"] = t

    base["awr"] = f(inputs["aw"])[:, :, 0]
    base["abr"] = f(inputs["ab"]).reshape(128, 1)

    # expert weights, K-major with bias rows
    ew = [f(inputs[f"ew{i+1}"]) for i in range(5)]
    eb = [f(inputs[f"eb{i+1}"]) for i in range(5)]
    km = lambda a: np.ascontiguousarray(a.transpose(1, 0, 2))  # [K, e, o]

    def aug(wB, bias, kpad=128):
        # [Kb, e, o] + bias row + zero rows -> [kpad, e, o]
        kb = wB.shape[0]
        out = np.zeros((kpad,) + wB.shape[1:], np.float32)
        out[0:kb] = wB
        out[kb] = bias
        return out

    for li, pre in ((0, "e1"), (1, "e2")):
        w = ew[li]
        base[pre + "A"] = km(w[:, 0:128, :]).astype(EW_NP)
        base[pre + "B"] = aug(km(w[:, 128:196, :]), eb[li]).astype(EW_NP)
    base["e3AB"] = np.stack(
        [km(ew[2][:, 0:128, :]),
         aug(km(ew[2][:, 128:196, :]), eb[2])], axis=2).astype(EW_NP)
    base["e4"] = aug(km(ew[3]), eb[3]).astype(EW_NP)
    base["e5"] = aug(km(ew[4]), eb[4], kpad=32).astype(EW_NP)

    fw1 = f(inputs["fw1"])
    # fw1p[o, e, n] = fw1[e*16+o, n]
    fw1p = np.ascontiguousarray(fw1.reshape(128, 16, FIN_DIMS[1]).transpose(1, 0, 2))
    base["fw1p"] = fw1p.astype(FW_NP)
    base["fb1r"] = f(inputs["fb1"]).reshape(1, -1).astype(FW_NP)
    base["fw2r"] = f(inputs["fw2"]).astype(FW_NP)
    base["fb2r"] = f(inputs["fb2"]).reshape(1, -1).astype(FW_NP)
    base["fw3r"] = f(inputs["fw3"]).astype(FW_NP)
    base["fb3r"] = f(inputs["fb3"]).reshape(1, -1).astype(FW_NP)
    base["onesrow"] = np.ones((1, 512), EW_NP)
    op = np.zeros((32, 512), np.float32); op[0] = 1.0
    base["onespad"] = op.astype(EW_NP)
    return base


def kernel(**inputs):
    global _BUILT
    if _BUILT is None:
        _BUILT = _build()
    nc = _BUILT
    base = _host_prep(inputs)
    x = np.asarray(inputs["x"], dtype=np.float32)
    in_maps = []
    for c in range(NCORES):
        m = dict(base)
        xc = x[c * BL:(c + 1) * BL]
        xp = np.zeros((BL, 3, H + 2, W + 2), np.float32)
        xp[:, :, 1:H + 1, 1:W + 1] = xc
        # xr[(ky,img,ch), r, c] = xp[img, ch, r+ky, c]
        xr = np.stack([xp[:, :, ky:ky + H, :] for ky in range(3)], axis=0)
        m["xr"] = np.ascontiguousarray(xr.reshape(36, H, W + 2)).astype(XR_NP)
        in_maps.append(m)
    res = run_bass_kernel_spmd(nc, in_maps, list(range(NCORES)))
    return np.concatenate([res.results[c]["y"] for c in range(NCORES)], axis=0)


if __name__ == "__main__":
    rng = np.random.default_rng(0)
    fake = {}
    # quick shape smoke with random inputs
    fake["x"] = rng.standard_normal((B, 3, H, W), dtype=np.float32)
    for i in range(4):
        cin, cout = CHANS[i], CHANS[i + 1]
        fake[f"cw{i+1}"] = rng.standard_normal((cout, cin, 3, 3), dtype=np.float32)
        fake[f"cb{i+1}"] = np.zeros(cout, np.float32)
    fake["aw"] = rng.standard_normal((NF, FLAT, 1), dtype=np.float32)
    fake["ab"] = np.zeros((NF, 1), np.float32)
    for i in range(5):
        di, do = EXP_DIMS[i], EXP_DIMS[i + 1]
        fake[f"ew{i+1}"] = rng.standard_normal((NF, di, do), dtype=np.float32)
        fake[f"eb{i+1}"] = np.zeros((NF, do), np.float32)
    for i in range(3):
        di, do = FIN_DIMS[i], FIN_DIMS[i + 1]
        fake[f"fw{i+1}"] = rng.standard_normal((di, do), dtype=np.float32)
        fake[f"fb{i+1}"] = np.zeros(do, np.float32)
    y = kernel(**fake)
    print("y", y.shape, y.dtype)

